# revision 18
# baseline (speedup 1.0000x reference)
"""Causal self-attention (B=2, T=2048, C=1024, H=16, HD=64, RoPE) on 8 TRN2 cores.

Sharding: data-parallel over batch (2 groups) x tensor-parallel over heads
(4 heads per core).  Each core computes qkv projection for its 4 heads, RoPE,
causal attention, and a partial output projection (row-split o_w); the host
sums the 4 partials per batch element and adds o_b.

On-device layout: activations are kept transposed ("feature, token"):
  x_lo/x_hi [C=1024, 1024]     host-transposed input halves, each contiguous
                               in DRAM so DMA descriptors are sequential reads
  Q^T,K^T [256, 2048]          d on partitions; per-head d order is permuted
                               [evens, odds] so RoPE works on partition halves
  V_aug  [T, 4*65]             per head 64 v-dims + a ones column that makes
                               the AV matmul emit the softmax row-sum for free
All matmul operands use float32r (TF32-like, 1 cyc/row at N>=256).

Schedule notes (v4): Scalar runs only Exp (+ table-free copies); RoPE fuses
qkv-bias via scalar_tensor_tensor on Vector with SP-queue DMA partition swaps;
softmax normalize broadcasts the row-sum on PE and uses the single-pass DVE
approx reciprocal.  Attention AV lags scores by TWO groups so the PE never
waits on Exp, scores own the big PSUM pool exclusively, and all projection /
output work is emitted interleaved into the attention head loops as PE filler.
"""

import numpy as np

import concourse.bass as bass
import concourse.mybir as mybir
import concourse.tile as tile
from concourse import bacc
from concourse.bass_utils import run_bass_kernel_spmd

F32 = mybir.dt.float32
F32R = mybir.dt.float32r
ADD = mybir.AluOpType.add
MULT = mybir.AluOpType.mult

B, T, C, H, HD = 2, 2048, 1024, 16, 64
HPC = 4          # heads per core
NCORES = 8
DH = HPC * HD    # 256 head dims per core
VW = HD + 1      # v block width incl. ones column
NKT = C // 128   # 8 k-tiles over the embedding dim
NTT = T // 128   # 16 token tiles of 128
NJ = T // 512    # 4 query-tile groups of 512
AVLAG = 2        # attention AV runs this many score-groups behind


def build_nc():
    nc = bacc.Bacc(None)

    x_lo_d = nc.declare_dram_parameter("x_lo", [C, 1024], F32R, isOutput=False)
    x_hi_d = nc.declare_dram_parameter("x_hi", [C, 1024], F32R, isOutput=False)
    w_qk = nc.declare_dram_parameter("w_qk", [C, 2 * DH], F32R, isOutput=False)
    qk_bias = nc.declare_dram_parameter("qk_bias", [128, 4], F32, isOutput=False)
    w_v = nc.declare_dram_parameter("w_v", [C + 1, HPC * VW], F32R, isOutput=False)
    o_wt = nc.declare_dram_parameter("o_wt", [DH, C], F32R, isOutput=False)
    cs_tab = nc.declare_dram_parameter("cs_tab", [32, T], F32R, isOutput=False)
    sn_tab = nc.declare_dram_parameter("sn_tab", [64, T], F32R, isOutput=False)
    tri = nc.declare_dram_parameter("tri", [128, 128], F32R, isOutput=False)
    ones = nc.declare_dram_parameter("ones", [1, 128], F32R, isOutput=False)
    y_t = nc.declare_dram_parameter("y_t", [C, T], F32, isOutput=True)

    with tile.TileContext(nc) as tc:
        with (
            tc.tile_pool(name="persist", bufs=1) as pp,
            tc.tile_pool(name="work", bufs=2) as wp,
            tc.tile_pool(name="pbig", bufs=2, space="PSUM") as ps_big,
            tc.tile_pool(name="pacc", bufs=2, space="PSUM") as ps_acc,
            tc.tile_pool(name="pmisc", bufs=2, space="PSUM") as ps_misc,
        ):
            # ---- persistent tensors ----
            qk_sb = [pp.tile([128, T], F32R, name=f"qk{m}", tag=f"qk{m}") for m in range(4)]
            v_sb = [pp.tile([128, HPC * VW], F32R, name=f"v{i}", tag=f"v{i}") for i in range(NTT)]
            o_final = [pp.tile([128, T], F32R, name=f"of{i}", tag=f"of{i}") for i in range(2)]
            w_qk_sb = [pp.tile([128, 2 * DH], F32R, name=f"wqk{i}", tag=f"wqk{i}") for i in range(NKT)]
            w_v_sb = [pp.tile([128, HPC * VW], F32R, name=f"wvt{i}", tag=f"wvt{i}") for i in range(NKT)]
            cs_sb = pp.tile([128, T], F32R, tag="cs")
            sn_sb = pp.tile([128, T], F32R, tag="sn")
            ow_sb = [pp.tile([128, C], F32R, name=f"ow{i}", tag=f"ow{i}") for i in range(2)]
            wv_bias = pp.tile([1, HPC * VW], F32R, tag="wvb")
            bias_sb = pp.tile([128, 4], F32, tag="bias")
            tri_sb = pp.tile([128, 128], F32R, tag="tri")
            ones_sb = pp.tile([1, 128], F32R, tag="ones")

            def rope(m, col0, ps):
                """qk_sb[m][:, col0:col0+512] = (ps+b)*cs + qswap((ps+b)*sn).

                ps is a [128, 512] PSUM slice; sn is host-negated on odd
                quarters so the rotation reduces to a 32-partition quarter
                swap (SP-queue SBUF-to-SBUF DMA) plus two fused fmas.
                """
                g = qk_sb[m][:, col0:col0 + 512]
                cs_c = cs_sb[:, col0:col0 + 512]
                sn_c = sn_sb[:, col0:col0 + 512]
                bm = bias_sb[:, m:m + 1]
                rb = wp.tile([128, 512], F32, tag="ropeB", bufs=2, name=f"rb{m}_{col0}")
                nc.vector.scalar_tensor_tensor(g, ps[:, :], bm, cs_c, ADD, MULT)
                nc.vector.scalar_tensor_tensor(rb[:], ps[:, :], bm, sn_c, ADD, MULT)
                # quarter-swap of rb accumulated straight into g by the DMA
                for q0 in range(0, 128, 64):
                    nc.gpsimd.dma_start(out=qk_sb[m][q0:q0 + 32, col0:col0 + 512],
                                        in_=rb[q0 + 32:q0 + 64, :], accum_op=ADD)
                    nc.gpsimd.dma_start(out=qk_sb[m][q0 + 32:q0 + 64, col0:col0 + 512],
                                        in_=rb[q0:q0 + 32, :], accum_op=ADD)

            def qk_proj(m, xs, jt, s):
                """One [128, 512] QK-projection block + RoPE, via ps_misc."""
                ps = ps_misc.tile([128, 512], F32, tag="misc", name=f"qkp{m}_{jt}{s}")
                for c in range(NKT):
                    nc.tensor.matmul(
                        ps[:], w_qk_sb[c][:, 128 * m:128 * m + 128],
                        xs[c][:, 512 * s:512 * s + 512],
                        start=(c == 0), stop=(c == NKT - 1),
                    )
                rope(m, 1024 * jt + 512 * s, ps)

            def v_proj(xs, trange):
                for t in trange:
                    ps = ps_misc.tile([128, HPC * VW], F32, tag="misc", name=f"vp{t}")
                    for c in range(NKT):
                        nc.tensor.matmul(
                            ps[:], xs[c][:, 128 * (t % 8):128 * (t % 8) + 128],
                            w_v_sb[c][:], start=(c == 0), stop=False,
                        )
                    nc.tensor.matmul(ps[:], ones_sb[:1, :], wv_bias[:1, :],
                                     start=False, stop=True)
                    # Copy activation is table-free, so Scalar stays Exp-only
                    nc.scalar.copy(v_sb[t][:], ps[:])

            def emit_normalize(h, J, po):
                """o_final <- po[0:64] * (1/rowsum) broadcast over rows."""
                pb = 64 * (h % 2)
                # rowsum -> SBUF (f32r), broadcast over 64 rows on PE, then a
                # single-pass approx reciprocal (~18 bits); the final multiply
                # reads po as its only PSUM operand.
                rsum = wp.tile([1, 512], F32R, tag="rsum", bufs=1, name=f"rs{h}_{J}")
                nc.vector.tensor_copy(rsum[:1, :], po[64:65, :])
                pr = ps_misc.tile([64, 512], F32, tag="misc", name=f"pr{h}_{J}")
                nc.tensor.matmul(pr[:], ones_sb[:1, :64], rsum[:1, :],
                                 start=True, stop=True)
                inv = wp.tile([64, 512], F32, tag="inv", bufs=1, name=f"inv{h}_{J}")
                nc.vector.reciprocal_approx_fast(inv[:], pr[:, :])
                nc.vector.tensor_mul(
                    o_final[h // 2][pb:pb + 64, 512 * J:512 * J + 512],
                    po[0:64, :], inv[:],
                )

            def attn_phase(J, fillers=()):
                fill = list(fillers)
                pending_norm = None
                for h in range(HPC):
                    qt = qk_sb[0 if h < 2 else 1]
                    kt = qk_sb[2 if h < 2 else 3]
                    pb = 64 * (h % 2)
                    tq0 = 512 * J
                    po = ps_acc.tile([65, 512], F32, tag="acc", name=f"po{h}_{J}")
                    n_i = 4 * J + 4

                    def emit_av(p_sb, i0):
                        for s in range(2):
                            i = i0 + s
                            d = max(0, 128 * (i - 4 * J))
                            nc.tensor.matmul(
                                po[:, d:512],
                                v_sb[i][:, VW * h:VW * h + VW],
                                p_sb[:, 512 * s + d:512 * s + 512],
                                start=(i == 0), stop=(i == n_i - 1),
                                skip_group_check=True,
                            )

                    pending_av = []  # AV lags AVLAG groups so PE never waits on exp
                    for i0 in range(0, n_i, 2):
                        ps = ps_big.tile([128, 1024], F32, tag="big", name=f"s{h}_{J}_{i0}")
                        for s in range(2):
                            i = i0 + s
                            nc.tensor.matmul(
                                ps[:, 512 * s:512 * s + 512],
                                kt[pb:pb + 64, 128 * i:128 * i + 128],
                                qt[pb:pb + 64, tq0:tq0 + 512],
                                start=True, stop=True,
                            )
                        p_sb = wp.tile([128, 1024], F32R, tag="p", bufs=2 + AVLAG,
                                       name=f"p{h}_{J}_{i0}")
                        nc.scalar.activation(
                            p_sb[:], ps[:], mybir.ActivationFunctionType.Exp,
                            scale=float(HD) ** -0.5,
                        )
                        for s in range(2):
                            i = i0 + s
                            if i >= 4 * J:  # diagonal tile: mask the triangle
                                d = 128 * (i - 4 * J)
                                nc.gpsimd.tensor_mul(
                                    p_sb[:, 512 * s + d:512 * s + d + 128],
                                    p_sb[:, 512 * s + d:512 * s + d + 128],
                                    tri_sb[:],
                                )
                        pending_av.append((p_sb, i0))
                        if len(pending_av) > AVLAG:
                            emit_av(*pending_av.pop(0))
                    for pa in pending_av:
                        emit_av(*pa)
                    if pending_norm is not None:
                        emit_normalize(*pending_norm)
                    pending_norm = (h, J, po)
                    if fill:
                        f = fill.pop(0)
                        if f is not None:
                            f()
                emit_normalize(*pending_norm)

            def out_proj(J, erange, alternate=False):
                for e in erange:
                    ps = ps_misc.tile([128, 512], F32, tag="misc", name=f"yp{J}_{e}")
                    for ct in range(2):
                        nc.tensor.matmul(
                            ps[:], ow_sb[ct][:, 128 * e:128 * e + 128],
                            o_final[ct][:, 512 * J:512 * J + 512],
                            start=(ct == 0), stop=(ct == 1),
                        )
                    y_sb = wp.tile([128, 512], F32, tag="ysb", bufs=2, name=f"y{J}_{e}")
                    if alternate and e % 2:
                        nc.scalar.copy(y_sb[:], ps[:])
                    else:
                        nc.vector.tensor_copy(y_sb[:], ps[:])
                    nc.sync.dma_start(
                        out=y_t[128 * e:128 * e + 128, 512 * J:512 * J + 512],
                        in_=y_sb[:],
                    )

            # ---------------- schedule ----------------
            with tc.tile_pool(name="xhi", bufs=1) as pxb:
                x_hi = [pxb.tile([128, 1024], F32R, name=f"xh{i}", tag=f"xh{i}")
                        for i in range(NKT)]
                with tc.tile_pool(name="xlo", bufs=1) as pxa:
                    x_lo = [pxa.tile([128, 1024], F32R, name=f"xl{i}", tag=f"xl{i}")
                            for i in range(NKT)]
                    # ---- DMA preamble, priority-ordered so PE can start early.
                    for i in range(NKT):
                        nc.sync.dma_start(out=w_qk_sb[i][:], in_=w_qk[128 * i:128 * i + 128, :])
                        if i == 0:
                            nc.sync.dma_start(out=x_lo[0][:, 0:512], in_=x_lo_d[0:128, 0:512])
                            nc.sync.dma_start(out=x_lo[0][:, 512:1024], in_=x_lo_d[0:128, 512:1024])
                        else:
                            nc.sync.dma_start(out=x_lo[i][:], in_=x_lo_d[128 * i:128 * i + 128, :])
                        if i == 3:
                            # small tables, needed by the first RoPE
                            nc.sync.dma_start(out=bias_sb[:], in_=qk_bias[:, :])
                            nc.sync.dma_start(out=cs_sb[0:32, :], in_=cs_tab[:, :])
                            nc.sync.dma_start(out=sn_sb[0:64, :], in_=sn_tab[:, :])
                            for r in (32, 64, 96):
                                nc.sync.dma_start(out=cs_sb[r:r + 32, :], in_=cs_sb[0:32, :])
                            nc.sync.dma_start(out=sn_sb[64:128, :], in_=sn_sb[0:64, :])
                            nc.sync.dma_start(out=tri_sb[:], in_=tri[:, :])
                            nc.sync.dma_start(out=ones_sb[:1, :], in_=ones[:, :])
                            nc.sync.dma_start(out=wv_bias[:1, :], in_=w_v[C:C + 1, :])
                    for i in range(NKT):
                        nc.sync.dma_start(out=w_v_sb[i][:], in_=w_v[128 * i:128 * i + 128, :])
                    for i in range(NKT):
                        nc.sync.dma_start(out=x_hi[i][:], in_=x_hi_d[128 * i:128 * i + 128, :])
                    for i in range(2):
                        nc.sync.dma_start(out=ow_sb[i][:], in_=o_wt[128 * i:128 * i + 128, :])

                    # pre-load the Exp activation table while Scalar is idle
                    warm = wp.tile([128, 2], F32, tag="inv", bufs=1, name="warm")
                    nc.scalar.activation(warm[:], tri_sb[:, 0:2],
                                         mybir.ActivationFunctionType.Exp)

                    # ---- P0: QK projection for tokens 0..1023, c-outer across
                    # all four m-blocks at once (8 independent PSUM-bank
                    # groups spanning all three pools) so the PE streams
                    # continuously while x/w tiles arrive, and no m-block
                    # waits on another's RoPE consumption.
                    pss = {
                        0: ps_big.tile([128, 1024], F32, tag="big", name="qkp0_0"),
                        2: ps_big.tile([128, 1024], F32, tag="big", name="qkp2_0"),
                        1: [ps_acc.tile([128, 512], F32, tag="acc", name=f"qkp1_0{s}")
                            for s in range(2)],
                        3: [ps_misc.tile([128, 512], F32, tag="misc", name=f"qkp3_0{s}")
                            for s in range(2)],
                    }
                    for c in range(NKT):
                        for m in (0, 2, 1, 3):
                            for s in range(2):
                                dst = (pss[m][:, 512 * s:512 * s + 512]
                                       if m in (0, 2) else pss[m][s][:, :])
                                nc.tensor.matmul(
                                    dst,
                                    w_qk_sb[c][:, 128 * m:128 * m + 128],
                                    x_lo[c][:, 512 * s:512 * s + 512],
                                    start=(c == 0), stop=(c == NKT - 1),
                                    skip_group_check=True,
                                )
                    for m in (0, 2, 1, 3):
                        for s in range(2):
                            src = (pss[m][:, 512 * s:512 * s + 512]
                                   if m in (0, 2) else pss[m][s][:, :])
                            rope(m, 512 * s, src)
                    v_proj(x_lo, range(0, 4))

                    # fillers matched to the DMA arrival order: x_lo work in
                    # J0, x_hi work (qk1/vp) from J1 once x_hi has streamed in.
                    def qk1(m):
                        def f():
                            for s in range(2):
                                qk_proj(m, x_hi, 1, s)
                        return f

                    def vpl(t0):
                        return lambda: v_proj(x_lo, range(t0, t0 + 2))

                    def vph(t0):
                        return lambda: v_proj(x_hi, range(t0, t0 + 2))

                    def op(J, e0):
                        return lambda: out_proj(J, range(e0, e0 + 4))

                    def qk1v(m0, m1):
                        def f():
                            qk1(m0)()
                            qk1(m1)()
                        return f

                    def vph4():
                        return lambda: v_proj(x_hi, range(8, 12))

                    attn_phase(0, [vpl(4), vpl(6), None, None])
                # x_lo freed after attention J0
                attn_phase(1, [qk1(0), qk1(2), qk1v(1, 3), vph4()])
                attn_phase(2, [vph(12), vph(14), op(0, 0), op(0, 4)])
            attn_phase(3, [op(1, 0), op(1, 4), op(2, 0), op(2, 4)])
            out_proj(3, range(NKT), alternate=True)

    nc.finalize()
    return nc


def _round_tf32(x):
    b = np.ascontiguousarray(x, dtype=np.float32).view(np.uint32)
    lsb = (b >> np.uint32(13)) & np.uint32(1)
    r = (b + np.uint32(0x0FFF) + lsb) & np.uint32(0xFFFFE000)
    return r.view(np.float32)


_NC = None


def _perm128():
    """d order within a 2-head qk tile: per head [evens(32), odds(32)]."""
    ev = np.arange(0, 64, 2)
    od = np.arange(1, 64, 2)
    return np.concatenate([ev, od, 64 + ev, 64 + od])


def _host_inputs(x, freqs_cos, freqs_sin, qkv_w, qkv_b, o_w):
    """Build the 8 per-core input maps."""
    x = np.asarray(x, np.float32)
    qkv_w = np.asarray(qkv_w, np.float32)
    qkv_b = np.asarray(qkv_b, np.float32)
    o_w = np.asarray(o_w, np.float32)
    cosT = np.asarray(freqs_cos, np.float32).T  # [32, T]
    sinT = np.asarray(freqs_sin, np.float32).T
    cs = _round_tf32(cosT)  # [32, T]; replicated to 128 partitions on-device
    # per-head layout [evens(32), odds(32)]: odd quarters get -sin so rope
    # reduces to g*cs + quarterswap(g*sn)
    sn = _round_tf32(np.concatenate([sinT, -sinT]))  # [64, T]; replicated 2x
    p, g_idx = np.mgrid[0:128, 0:128]
    tri = (g_idx >= p).astype(np.float32)
    ones = np.ones((1, 128), np.float32)
    perm = _perm128()

    maps = []
    for core in range(NCORES):
        b, grp = divmod(core, NCORES // B)
        H0 = HPC * grp
        # permuted global q/k row indices for this core's heads (per 2-head tile)
        q_rows = np.concatenate([64 * (H0 + 2 * t) + perm for t in range(HPC // 2)])
        k_rows = C + q_rows
        v_rows = 2 * C + np.arange(64 * H0, 64 * H0 + DH)
        w_qk = _round_tf32(qkv_w[np.concatenate([q_rows, k_rows])].T)  # [C, 512]
        qk_bias = np.stack(
            [qkv_b[q_rows[:128]], qkv_b[q_rows[128:]],
             qkv_b[k_rows[:128]], qkv_b[k_rows[128:]]], axis=1)  # [128, 4]
        w_v = np.zeros((C + 1, HPC * VW), np.float32)
        for h in range(HPC):
            rows = v_rows[64 * h:64 * h + 64]
            w_v[:C, VW * h:VW * h + HD] = qkv_w[rows].T
            w_v[C, VW * h:VW * h + HD] = qkv_b[rows]
            w_v[C, VW * h + HD] = 1.0
        o_wt = _round_tf32(o_w[:, 64 * H0:64 * H0 + DH].T)  # [DH, C]
        xt = _round_tf32(x[b].T)
        maps.append({
            "x_lo": np.ascontiguousarray(xt[:, 0:1024]),
            "x_hi": np.ascontiguousarray(xt[:, 1024:2048]),
            "w_qk": w_qk,
            "qk_bias": qk_bias.astype(np.float32),
            "w_v": _round_tf32(w_v),
            "o_wt": o_wt,
            "cs_tab": cs,
            "sn_tab": sn,
            "tri": tri,
            "ones": ones,
        })
    return maps


def kernel(x, freqs_cos, freqs_sin, qkv_w, qkv_b, o_w, o_b, _trace=False, _tmpdir=None):
    global _NC
    if _NC is None:
        _NC = build_nc()
    maps = _host_inputs(x, freqs_cos, freqs_sin, qkv_w, qkv_b, o_w)
    kw = {}
    if _trace:
        kw = dict(trace=True, tmpdir=_tmpdir)
    res = run_bass_kernel_spmd(_NC, maps, list(range(NCORES)), **kw)
    o_b = np.asarray(o_b, np.float32)
    out = np.empty((B, T, C), np.float32)
    g = NCORES // B
    for b in range(B):
        acc = res.results[g * b]["y_t"].astype(np.float32)
        for j in range(1, g):
            acc = acc + res.results[g * b + j]["y_t"]
        out[b] = acc.T + o_b
    kernel._last = res
    return out


# revision 20
# speedup vs baseline: 1.1602x; 1.1602x over previous
"""Causal self-attention (B=2, T=2048, C=1024, H=16, HD=64, RoPE) on 8 TRN2 cores.

Sharding: data-parallel over batch (2 groups) x tensor-parallel over heads
(4 heads per core).  Each core computes qkv projection for its 4 heads, RoPE,
causal attention, and a partial output projection (row-split o_w); the host
sums the 4 partials per batch element and adds o_b.

On-device layout: activations are kept transposed ("feature, token"):
  x_lo/x_hi [C=1024, 1024]     host-transposed input halves, each contiguous
                               in DRAM so DMA descriptors are sequential reads
  Q^T,K^T [256, 2048]          d on partitions; per-head d order is permuted
                               [evens, odds] so RoPE works on partition halves
  V_aug  [T, 4*65]             per head 64 v-dims + a ones column that makes
                               the AV matmul emit the softmax row-sum for free
All matmul operands use float32r (TF32-like, 1 cyc/row at N>=256).

Schedule notes (v4): Scalar runs only Exp (+ table-free copies); RoPE fuses
qkv-bias via scalar_tensor_tensor on Vector with SP-queue DMA partition swaps;
softmax normalize broadcasts the row-sum on PE and uses the single-pass DVE
approx reciprocal.  Attention AV lags scores by TWO groups so the PE never
waits on Exp, scores own the big PSUM pool exclusively, and all projection /
output work is emitted interleaved into the attention head loops as PE filler.
"""

import numpy as np

import concourse.bass as bass
import concourse.mybir as mybir
import concourse.tile as tile
from concourse import bacc
from concourse.bass_utils import run_bass_kernel_spmd

F32 = mybir.dt.float32
F32R = mybir.dt.float32r
ADD = mybir.AluOpType.add
MULT = mybir.AluOpType.mult

B, T, C, H, HD = 2, 2048, 1024, 16, 64
HPC = 4          # heads per core
NCORES = 8
DH = HPC * HD    # 256 head dims per core
VW = HD + 1      # v block width incl. ones column
NKT = C // 128   # 8 k-tiles over the embedding dim
NTT = T // 128   # 16 token tiles of 128
NJ = T // 512    # 4 query-tile groups of 512
AVLAG = 2        # attention AV runs this many score-groups behind


def build_nc():
    nc = bacc.Bacc(None)

    x_lo_d = nc.declare_dram_parameter("x_lo", [C, 1024], F32R, isOutput=False)
    x_hi_d = nc.declare_dram_parameter("x_hi", [C, 1024], F32R, isOutput=False)
    w_qk = nc.declare_dram_parameter("w_qk", [C, 2 * DH], F32R, isOutput=False)
    qk_bias = nc.declare_dram_parameter("qk_bias", [128, 4], F32, isOutput=False)
    w_v = nc.declare_dram_parameter("w_v", [C + 1, HPC * VW], F32R, isOutput=False)
    o_wt = nc.declare_dram_parameter("o_wt", [DH, C], F32R, isOutput=False)
    cs_tab = nc.declare_dram_parameter("cs_tab", [32, T], F32R, isOutput=False)
    sn_tab = nc.declare_dram_parameter("sn_tab", [64, T], F32R, isOutput=False)
    tri = nc.declare_dram_parameter("tri", [128, 128], F32R, isOutput=False)
    ones = nc.declare_dram_parameter("ones", [1, 128], F32R, isOutput=False)
    y_t = nc.declare_dram_parameter("y_t", [C, T], F32, isOutput=True)

    with tile.TileContext(nc) as tc:
        with (
            tc.tile_pool(name="persist", bufs=1) as pp,
            tc.tile_pool(name="work", bufs=2) as wp,
            tc.tile_pool(name="pbig", bufs=2, space="PSUM") as ps_big,
            tc.tile_pool(name="pacc", bufs=2, space="PSUM") as ps_acc,
            tc.tile_pool(name="pmisc", bufs=2, space="PSUM") as ps_misc,
        ):
            # ---- persistent tensors ----
            qk_sb = [pp.tile([128, T], F32R, name=f"qk{m}", tag=f"qk{m}") for m in range(4)]
            v_sb = [pp.tile([128, HPC * VW], F32R, name=f"v{i}", tag=f"v{i}") for i in range(NTT)]
            o_final = [pp.tile([128, T], F32R, name=f"of{i}", tag=f"of{i}") for i in range(2)]
            w_qk_sb = [pp.tile([128, 2 * DH], F32R, name=f"wqk{i}", tag=f"wqk{i}") for i in range(NKT)]
            w_v_sb = [pp.tile([128, HPC * VW], F32R, name=f"wvt{i}", tag=f"wvt{i}") for i in range(NKT)]
            cs_sb = pp.tile([128, T], F32R, tag="cs")
            sn_sb = pp.tile([128, T], F32R, tag="sn")
            ow_sb = [pp.tile([128, C], F32R, name=f"ow{i}", tag=f"ow{i}") for i in range(2)]
            wv_bias = pp.tile([1, HPC * VW], F32R, tag="wvb")
            bias_sb = pp.tile([128, 4], F32, tag="bias")
            tri_sb = pp.tile([128, 128], F32R, tag="tri")
            ones_sb = pp.tile([1, 128], F32R, tag="ones")

            def rope(m, col0, ps):
                """qk_sb[m][:, col0:col0+512] = (ps+b)*cs + qswap((ps+b)*sn).

                ps is a [128, 512] PSUM slice; sn is host-negated on odd
                quarters so the rotation reduces to a 32-partition quarter
                swap (SP-queue SBUF-to-SBUF DMA) plus two fused fmas.
                """
                g = qk_sb[m][:, col0:col0 + 512]
                cs_c = cs_sb[:, col0:col0 + 512]
                sn_c = sn_sb[:, col0:col0 + 512]
                bm = bias_sb[:, m:m + 1]
                rb = wp.tile([128, 512], F32, tag="ropeB", bufs=2, name=f"rb{m}_{col0}")
                nc.vector.scalar_tensor_tensor(g, ps[:, :], bm, cs_c, ADD, MULT)
                nc.vector.scalar_tensor_tensor(rb[:], ps[:, :], bm, sn_c, ADD, MULT)
                # quarter-swap of rb accumulated straight into g by the DMA
                for q0 in range(0, 128, 64):
                    nc.gpsimd.dma_start(out=qk_sb[m][q0:q0 + 32, col0:col0 + 512],
                                        in_=rb[q0 + 32:q0 + 64, :], accum_op=ADD)
                    nc.gpsimd.dma_start(out=qk_sb[m][q0 + 32:q0 + 64, col0:col0 + 512],
                                        in_=rb[q0:q0 + 32, :], accum_op=ADD)

            def qk_proj(m, xs, jt, s):
                """One [128, 512] QK-projection block + RoPE, via ps_misc."""
                ps = ps_misc.tile([128, 512], F32, tag="misc", name=f"qkp{m}_{jt}{s}")
                for c in range(NKT):
                    nc.tensor.matmul(
                        ps[:], w_qk_sb[c][:, 128 * m:128 * m + 128],
                        xs[c][:, 512 * s:512 * s + 512],
                        start=(c == 0), stop=(c == NKT - 1),
                    )
                rope(m, 1024 * jt + 512 * s, ps)

            def v_proj(xs, trange):
                for t in trange:
                    ps = ps_misc.tile([128, HPC * VW], F32, tag="misc", name=f"vp{t}")
                    for c in range(NKT):
                        nc.tensor.matmul(
                            ps[:], xs[c][:, 128 * (t % 8):128 * (t % 8) + 128],
                            w_v_sb[c][:], start=(c == 0), stop=False,
                        )
                    nc.tensor.matmul(ps[:], ones_sb[:1, :], wv_bias[:1, :],
                                     start=False, stop=True)
                    # Copy activation is table-free, so Scalar stays Exp-only
                    nc.scalar.copy(v_sb[t][:], ps[:])

            def emit_normalize(h, J, po):
                """o_final <- po[0:64] * (1/rowsum) broadcast over rows."""
                pb = 64 * (h % 2)
                # rowsum -> SBUF (f32r), broadcast over 64 rows on PE, then a
                # single-pass approx reciprocal (~18 bits); the final multiply
                # reads po as its only PSUM operand.
                rsum = wp.tile([1, 512], F32R, tag="rsum", bufs=1, name=f"rs{h}_{J}")
                nc.vector.tensor_copy(rsum[:1, :], po[64:65, :])
                pr = ps_misc.tile([64, 512], F32, tag="misc", name=f"pr{h}_{J}")
                nc.tensor.matmul(pr[:], ones_sb[:1, :64], rsum[:1, :],
                                 start=True, stop=True)
                inv = wp.tile([64, 512], F32, tag="inv", bufs=1, name=f"inv{h}_{J}")
                nc.vector.reciprocal_approx_fast(inv[:], pr[:, :])
                nc.vector.tensor_mul(
                    o_final[h // 2][pb:pb + 64, 512 * J:512 * J + 512],
                    po[0:64, :], inv[:],
                )

            def attn_phase(J, fillers=()):
                fill = list(fillers)
                pending_norm = None
                for h in range(HPC):
                    qt = qk_sb[0 if h < 2 else 1]
                    kt = qk_sb[2 if h < 2 else 3]
                    pb = 64 * (h % 2)
                    tq0 = 512 * J
                    po = ps_acc.tile([65, 512], F32, tag="acc", name=f"po{h}_{J}")
                    n_i = 4 * J + 4

                    def emit_av(p_sb, i0):
                        for s in range(2):
                            i = i0 + s
                            d = max(0, 128 * (i - 4 * J))
                            nc.tensor.matmul(
                                po[:, d:512],
                                v_sb[i][:, VW * h:VW * h + VW],
                                p_sb[:, 512 * s + d:512 * s + 512],
                                start=(i == 0), stop=(i == n_i - 1),
                                skip_group_check=True,
                            )

                    pending_av = []  # AV lags AVLAG groups so PE never waits on exp
                    for i0 in range(0, n_i, 2):
                        ps = ps_big.tile([128, 1024], F32, tag="big", name=f"s{h}_{J}_{i0}")
                        for s in range(2):
                            i = i0 + s
                            nc.tensor.matmul(
                                ps[:, 512 * s:512 * s + 512],
                                kt[pb:pb + 64, 128 * i:128 * i + 128],
                                qt[pb:pb + 64, tq0:tq0 + 512],
                                start=True, stop=True,
                            )
                        p_sb = wp.tile([128, 1024], F32R, tag="p", bufs=2 + AVLAG,
                                       name=f"p{h}_{J}_{i0}")
                        nc.scalar.activation(
                            p_sb[:], ps[:], mybir.ActivationFunctionType.Exp,
                            scale=float(HD) ** -0.5,
                        )
                        for s in range(2):
                            i = i0 + s
                            if i >= 4 * J:  # diagonal tile: mask the triangle
                                d = 128 * (i - 4 * J)
                                nc.gpsimd.tensor_mul(
                                    p_sb[:, 512 * s + d:512 * s + d + 128],
                                    p_sb[:, 512 * s + d:512 * s + d + 128],
                                    tri_sb[:],
                                )
                        pending_av.append((p_sb, i0))
                        if len(pending_av) > AVLAG:
                            emit_av(*pending_av.pop(0))
                    for pa in pending_av:
                        emit_av(*pa)
                    if pending_norm is not None:
                        emit_normalize(*pending_norm)
                    pending_norm = (h, J, po)
                    if fill:
                        f = fill.pop(0)
                        if f is not None:
                            f()
                emit_normalize(*pending_norm)

            def out_proj(J, erange, alternate=False):
                for e in erange:
                    ps = ps_misc.tile([128, 512], F32, tag="misc", name=f"yp{J}_{e}")
                    for ct in range(2):
                        nc.tensor.matmul(
                            ps[:], ow_sb[ct][:, 128 * e:128 * e + 128],
                            o_final[ct][:, 512 * J:512 * J + 512],
                            start=(ct == 0), stop=(ct == 1),
                        )
                    y_sb = wp.tile([128, 512], F32, tag="ysb", bufs=2, name=f"y{J}_{e}")
                    if alternate and e % 2:
                        nc.scalar.copy(y_sb[:], ps[:])
                    else:
                        nc.vector.tensor_copy(y_sb[:], ps[:])
                    nc.sync.dma_start(
                        out=y_t[128 * e:128 * e + 128, 512 * J:512 * J + 512],
                        in_=y_sb[:],
                    )

            # ---------------- schedule ----------------
            with tc.tile_pool(name="xhi", bufs=1) as pxb:
                x_hi = [pxb.tile([128, 1024], F32R, name=f"xh{i}", tag=f"xh{i}")
                        for i in range(NKT)]
                with tc.tile_pool(name="xlo", bufs=1) as pxa:
                    x_lo = [pxa.tile([128, 1024], F32R, name=f"xl{i}", tag=f"xl{i}")
                            for i in range(NKT)]
                    # ---- DMA preamble, priority-ordered so PE can start early.
                    for i in range(NKT):
                        nc.sync.dma_start(out=w_qk_sb[i][:], in_=w_qk[128 * i:128 * i + 128, :])
                        if i == 0:
                            nc.sync.dma_start(out=x_lo[0][:, 0:512], in_=x_lo_d[0:128, 0:512])
                            nc.sync.dma_start(out=x_lo[0][:, 512:1024], in_=x_lo_d[0:128, 512:1024])
                        else:
                            nc.sync.dma_start(out=x_lo[i][:], in_=x_lo_d[128 * i:128 * i + 128, :])
                        if i == 3:
                            # small tables, needed by the first RoPE
                            nc.sync.dma_start(out=bias_sb[:], in_=qk_bias[:, :])
                            nc.sync.dma_start(out=cs_sb[0:32, :], in_=cs_tab[:, :])
                            nc.sync.dma_start(out=sn_sb[0:64, :], in_=sn_tab[:, :])
                            for r in (32, 64, 96):
                                nc.sync.dma_start(out=cs_sb[r:r + 32, :], in_=cs_sb[0:32, :])
                            nc.sync.dma_start(out=sn_sb[64:128, :], in_=sn_sb[0:64, :])
                            nc.sync.dma_start(out=tri_sb[:], in_=tri[:, :])
                            nc.sync.dma_start(out=ones_sb[:1, :], in_=ones[:, :])
                            nc.sync.dma_start(out=wv_bias[:1, :], in_=w_v[C:C + 1, :])
                    for i in range(NKT):
                        nc.sync.dma_start(out=w_v_sb[i][:], in_=w_v[128 * i:128 * i + 128, :])
                    for i in range(NKT):
                        nc.sync.dma_start(out=x_hi[i][:], in_=x_hi_d[128 * i:128 * i + 128, :])
                    for i in range(2):
                        nc.sync.dma_start(out=ow_sb[i][:], in_=o_wt[128 * i:128 * i + 128, :])

                    # pre-load the Exp activation table while Scalar is idle
                    warm = wp.tile([128, 2], F32, tag="inv", bufs=1, name="warm")
                    nc.scalar.activation(warm[:], tri_sb[:, 0:2],
                                         mybir.ActivationFunctionType.Exp)

                    # ---- P0: QK projection for tokens 0..1023, c-outer across
                    # all four m-blocks at once (8 independent PSUM-bank
                    # groups spanning all three pools) so the PE streams
                    # continuously while x/w tiles arrive, and no m-block
                    # waits on another's RoPE consumption.
                    pss = {
                        0: ps_big.tile([128, 1024], F32, tag="big", name="qkp0_0"),
                        2: ps_big.tile([128, 1024], F32, tag="big", name="qkp2_0"),
                        1: [ps_acc.tile([128, 512], F32, tag="acc", name=f"qkp1_0{s}")
                            for s in range(2)],
                        3: [ps_misc.tile([128, 512], F32, tag="misc", name=f"qkp3_0{s}")
                            for s in range(2)],
                    }
                    for c in range(NKT):
                        for m in (0, 2, 1, 3):
                            for s in range(2):
                                dst = (pss[m][:, 512 * s:512 * s + 512]
                                       if m in (0, 2) else pss[m][s][:, :])
                                nc.tensor.matmul(
                                    dst,
                                    w_qk_sb[c][:, 128 * m:128 * m + 128],
                                    x_lo[c][:, 512 * s:512 * s + 512],
                                    start=(c == 0), stop=(c == NKT - 1),
                                    skip_group_check=True,
                                )
                    for m in (0, 2, 1, 3):
                        for s in range(2):
                            src = (pss[m][:, 512 * s:512 * s + 512]
                                   if m in (0, 2) else pss[m][s][:, :])
                            rope(m, 512 * s, src)
                    v_proj(x_lo, range(0, 4))

                    # fillers matched to the DMA arrival order: x_lo work in
                    # J0, x_hi work (qk1/vp) from J1 once x_hi has streamed in.
                    def qk1(m):
                        def f():
                            # keep the Tile scheduler from hoisting x_hi work
                            # ahead of attention J0 (x_hi streams in late)
                            with tc.tile_wait_until(0.034):
                                for s in range(2):
                                    qk_proj(m, x_hi, 1, s)
                        return f

                    def vpl(t0):
                        return lambda: v_proj(x_lo, range(t0, t0 + 2))

                    def vph(t0):
                        def f():
                            with tc.tile_wait_until(0.034):
                                v_proj(x_hi, range(t0, t0 + 2))
                        return f

                    def vph4():
                        def f():
                            with tc.tile_wait_until(0.034):
                                v_proj(x_hi, range(8, 12))
                        return f

                    def op(J, e0):
                        return lambda: out_proj(J, range(e0, e0 + 4))

                    def qk1v(m0, m1):
                        def f():
                            qk1(m0)()
                            qk1(m1)()
                        return f

                    attn_phase(0, [vpl(4), vpl(6), None, None])
                # x_lo freed after attention J0
                attn_phase(1, [qk1(0), qk1(2), qk1v(1, 3), vph4()])
                attn_phase(2, [vph(12), vph(14), op(0, 0), op(0, 4)])
            attn_phase(3, [op(1, 0), op(1, 4), op(2, 0), op(2, 4)])
            out_proj(3, range(NKT), alternate=True)

    nc.finalize()
    return nc


def _round_tf32(x):
    b = np.ascontiguousarray(x, dtype=np.float32).view(np.uint32)
    lsb = (b >> np.uint32(13)) & np.uint32(1)
    r = (b + np.uint32(0x0FFF) + lsb) & np.uint32(0xFFFFE000)
    return r.view(np.float32)


_NC = None


def _perm128():
    """d order within a 2-head qk tile: per head [evens(32), odds(32)]."""
    ev = np.arange(0, 64, 2)
    od = np.arange(1, 64, 2)
    return np.concatenate([ev, od, 64 + ev, 64 + od])


def _host_inputs(x, freqs_cos, freqs_sin, qkv_w, qkv_b, o_w):
    """Build the 8 per-core input maps."""
    x = np.asarray(x, np.float32)
    qkv_w = np.asarray(qkv_w, np.float32)
    qkv_b = np.asarray(qkv_b, np.float32)
    o_w = np.asarray(o_w, np.float32)
    cosT = np.asarray(freqs_cos, np.float32).T  # [32, T]
    sinT = np.asarray(freqs_sin, np.float32).T
    cs = _round_tf32(cosT)  # [32, T]; replicated to 128 partitions on-device
    # per-head layout [evens(32), odds(32)]: odd quarters get -sin so rope
    # reduces to g*cs + quarterswap(g*sn)
    sn = _round_tf32(np.concatenate([sinT, -sinT]))  # [64, T]; replicated 2x
    p, g_idx = np.mgrid[0:128, 0:128]
    tri = (g_idx >= p).astype(np.float32)
    ones = np.ones((1, 128), np.float32)
    perm = _perm128()

    maps = []
    for core in range(NCORES):
        b, grp = divmod(core, NCORES // B)
        H0 = HPC * grp
        # permuted global q/k row indices for this core's heads (per 2-head tile)
        q_rows = np.concatenate([64 * (H0 + 2 * t) + perm for t in range(HPC // 2)])
        k_rows = C + q_rows
        v_rows = 2 * C + np.arange(64 * H0, 64 * H0 + DH)
        w_qk = _round_tf32(qkv_w[np.concatenate([q_rows, k_rows])].T)  # [C, 512]
        qk_bias = np.stack(
            [qkv_b[q_rows[:128]], qkv_b[q_rows[128:]],
             qkv_b[k_rows[:128]], qkv_b[k_rows[128:]]], axis=1)  # [128, 4]
        w_v = np.zeros((C + 1, HPC * VW), np.float32)
        for h in range(HPC):
            rows = v_rows[64 * h:64 * h + 64]
            w_v[:C, VW * h:VW * h + HD] = qkv_w[rows].T
            w_v[C, VW * h:VW * h + HD] = qkv_b[rows]
            w_v[C, VW * h + HD] = 1.0
        o_wt = _round_tf32(o_w[:, 64 * H0:64 * H0 + DH].T)  # [DH, C]
        xt = _round_tf32(x[b].T)
        maps.append({
            "x_lo": np.ascontiguousarray(xt[:, 0:1024]),
            "x_hi": np.ascontiguousarray(xt[:, 1024:2048]),
            "w_qk": w_qk,
            "qk_bias": qk_bias.astype(np.float32),
            "w_v": _round_tf32(w_v),
            "o_wt": o_wt,
            "cs_tab": cs,
            "sn_tab": sn,
            "tri": tri,
            "ones": ones,
        })
    return maps


def kernel(x, freqs_cos, freqs_sin, qkv_w, qkv_b, o_w, o_b, _trace=False, _tmpdir=None):
    global _NC
    if _NC is None:
        _NC = build_nc()
    maps = _host_inputs(x, freqs_cos, freqs_sin, qkv_w, qkv_b, o_w)
    kw = {}
    if _trace:
        kw = dict(trace=True, tmpdir=_tmpdir)
    res = run_bass_kernel_spmd(_NC, maps, list(range(NCORES)), **kw)
    o_b = np.asarray(o_b, np.float32)
    out = np.empty((B, T, C), np.float32)
    g = NCORES // B
    for b in range(B):
        acc = res.results[g * b]["y_t"].astype(np.float32)
        for j in range(1, g):
            acc = acc + res.results[g * b + j]["y_t"]
        out[b] = acc.T + o_b
    kernel._last = res
    return out


# revision 25
# speedup vs baseline: 1.2861x; 1.1085x over previous
"""Causal self-attention (B=2, T=2048, C=1024, H=16, HD=64, RoPE) on 8 TRN2 cores.

Sharding: data-parallel over batch (2 groups) x tensor-parallel over heads
(4 heads per core).  Each core computes qkv projection for its 4 heads, RoPE,
causal attention, and a partial output projection (row-split o_w); the host
sums the 4 partials per batch element and adds o_b.

On-device layout: activations are kept transposed ("feature, token"):
  x_lo/x_hi [C=1024, 1024]     host-transposed input halves, each contiguous
                               in DRAM so DMA descriptors are sequential reads
  Q^T,K^T [256, 2048]          d on partitions; per-head d order is permuted
                               [evens, odds] so RoPE works on partition halves
  V_aug  [T, 4*65]             per head 64 v-dims + a ones column that makes
                               the AV matmul emit the softmax row-sum for free
All matmul operands use float32r (TF32-like, 1 cyc/row at N>=256).

Schedule notes (v4): Scalar runs only Exp (+ table-free copies); RoPE fuses
qkv-bias via scalar_tensor_tensor on Vector with SP-queue DMA partition swaps;
softmax normalize broadcasts the row-sum on PE and uses the single-pass DVE
approx reciprocal.  Attention AV lags scores by TWO groups so the PE never
waits on Exp, scores own the big PSUM pool exclusively, and all projection /
output work is emitted interleaved into the attention head loops as PE filler.
"""

import numpy as np

import concourse.bass as bass
import concourse.mybir as mybir
import concourse.tile as tile
from concourse import bacc
from concourse.bass_utils import run_bass_kernel_spmd

F32 = mybir.dt.float32
F32R = mybir.dt.float32r
ADD = mybir.AluOpType.add
MULT = mybir.AluOpType.mult

B, T, C, H, HD = 2, 2048, 1024, 16, 64
HPC = 4          # heads per core
NCORES = 8
DH = HPC * HD    # 256 head dims per core
VW = HD + 1      # v block width incl. ones column
NKT = C // 128   # 8 k-tiles over the embedding dim
NTT = T // 128   # 16 token tiles of 128
NJ = T // 512    # 4 query-tile groups of 512
AVLAG = 2        # attention AV runs this many score-groups behind


def build_nc():
    nc = bacc.Bacc(None)

    x_ls0_d = nc.declare_dram_parameter("x_ls0", [C, 512], F32R, isOutput=False)
    x_ls1_d = nc.declare_dram_parameter("x_ls1", [C, 512], F32R, isOutput=False)
    x_hi_d = nc.declare_dram_parameter("x_hi", [C, 1024], F32R, isOutput=False)
    w_qk = nc.declare_dram_parameter("w_qk", [C, 2 * DH], F32R, isOutput=False)
    qk_bias = nc.declare_dram_parameter("qk_bias", [128, 4], F32, isOutput=False)
    w_v = nc.declare_dram_parameter("w_v", [C + 1, HPC * VW], F32R, isOutput=False)
    o_wt = nc.declare_dram_parameter("o_wt", [DH, C], F32R, isOutput=False)
    cs_tab = nc.declare_dram_parameter("cs_tab", [32, T], F32R, isOutput=False)
    sn_tab = nc.declare_dram_parameter("sn_tab", [64, T], F32R, isOutput=False)
    tri = nc.declare_dram_parameter("tri", [128, 128], F32R, isOutput=False)
    ones = nc.declare_dram_parameter("ones", [1, 128], F32R, isOutput=False)
    y_t = nc.declare_dram_parameter("y_t", [C, T], F32, isOutput=True)

    with tile.TileContext(nc) as tc:
        with (
            tc.tile_pool(name="persist", bufs=1) as pp,
            tc.tile_pool(name="work", bufs=2) as wp,
            tc.tile_pool(name="pbig", bufs=2, space="PSUM") as ps_big,
            tc.tile_pool(name="pacc", bufs=2, space="PSUM") as ps_acc,
            tc.tile_pool(name="pmisc", bufs=2, space="PSUM") as ps_misc,
        ):
            # ---- persistent tensors ----
            qk_sb = [pp.tile([128, T], F32R, name=f"qk{m}", tag=f"qk{m}") for m in range(4)]
            v_sb = [pp.tile([128, HPC * VW], F32R, name=f"v{i}", tag=f"v{i}") for i in range(NTT)]
            o_final = [pp.tile([128, T], F32R, name=f"of{i}", tag=f"of{i}") for i in range(2)]
            w_qk_sb = [pp.tile([128, 2 * DH], F32R, name=f"wqk{i}", tag=f"wqk{i}") for i in range(NKT)]
            w_v_sb = [pp.tile([128, HPC * VW], F32R, name=f"wvt{i}", tag=f"wvt{i}") for i in range(NKT)]
            cs_sb = pp.tile([128, T], F32R, tag="cs")
            sn_sb = pp.tile([128, T], F32R, tag="sn")
            ow_sb = [pp.tile([128, C], F32R, name=f"ow{i}", tag=f"ow{i}") for i in range(2)]
            wv_bias = pp.tile([1, HPC * VW], F32R, tag="wvb")
            bias_sb = pp.tile([128, 4], F32, tag="bias")
            tri_sb = pp.tile([128, 128], F32R, tag="tri")
            ones_sb = pp.tile([1, 128], F32R, tag="ones")

            def rope(m, col0, ps):
                """qk_sb[m][:, col0:col0+512] = (ps+b)*cs + qswap((ps+b)*sn).

                ps is a [128, 512] PSUM slice; sn is host-negated on odd
                quarters so the rotation reduces to a 32-partition quarter
                swap (SP-queue SBUF-to-SBUF DMA) plus two fused fmas.
                """
                g = qk_sb[m][:, col0:col0 + 512]
                cs_c = cs_sb[:, col0:col0 + 512]
                sn_c = sn_sb[:, col0:col0 + 512]
                bm = bias_sb[:, m:m + 1]
                rb = wp.tile([128, 512], F32, tag="ropeB", bufs=2, name=f"rb{m}_{col0}")
                nc.vector.scalar_tensor_tensor(g, ps[:, :], bm, cs_c, ADD, MULT)
                nc.vector.scalar_tensor_tensor(rb[:], ps[:, :], bm, sn_c, ADD, MULT)
                # quarter-swap of rb accumulated straight into g by the DMA
                for q0 in range(0, 128, 64):
                    nc.gpsimd.dma_start(out=qk_sb[m][q0:q0 + 32, col0:col0 + 512],
                                        in_=rb[q0 + 32:q0 + 64, :], accum_op=ADD)
                    nc.gpsimd.dma_start(out=qk_sb[m][q0 + 32:q0 + 64, col0:col0 + 512],
                                        in_=rb[q0:q0 + 32, :], accum_op=ADD)

            def qk_proj(m, xs, jt, s):
                """One [128, 512] QK-projection block + RoPE, via ps_misc."""
                ps = ps_misc.tile([128, 512], F32, tag="misc", name=f"qkp{m}_{jt}{s}")
                for c in range(NKT):
                    nc.tensor.matmul(
                        ps[:], w_qk_sb[c][:, 128 * m:128 * m + 128],
                        xs[c][:, 512 * s:512 * s + 512],
                        start=(c == 0), stop=(c == NKT - 1),
                    )
                rope(m, 1024 * jt + 512 * s, ps)

            def v_proj(xs, trange):
                for t in trange:
                    ps = ps_misc.tile([128, HPC * VW], F32, tag="misc", name=f"vp{t}")
                    for c in range(NKT):
                        nc.tensor.matmul(
                            ps[:], xs[c][:, 128 * (t % 8):128 * (t % 8) + 128],
                            w_v_sb[c][:], start=(c == 0), stop=False,
                        )
                    nc.tensor.matmul(ps[:], ones_sb[:1, :], wv_bias[:1, :],
                                     start=False, stop=True)
                    # Copy activation is table-free, so Scalar stays Exp-only
                    nc.scalar.copy(v_sb[t][:], ps[:])

            def emit_normalize(h, J, po):
                """o_final <- po[0:64] * (1/rowsum) broadcast over rows."""
                pb = 64 * (h % 2)
                # rowsum -> SBUF (f32r), broadcast over 64 rows on PE, then a
                # single-pass approx reciprocal (~18 bits); the final multiply
                # reads po as its only PSUM operand.
                rsum = wp.tile([1, 512], F32R, tag="rsum", bufs=1, name=f"rs{h}_{J}")
                nc.vector.tensor_copy(rsum[:1, :], po[64:65, :])
                pr = ps_misc.tile([64, 512], F32, tag="misc", name=f"pr{h}_{J}")
                nc.tensor.matmul(pr[:], ones_sb[:1, :64], rsum[:1, :],
                                 start=True, stop=True)
                inv = wp.tile([64, 512], F32, tag="inv", bufs=1, name=f"inv{h}_{J}")
                nc.vector.reciprocal_approx_fast(inv[:], pr[:, :])
                nc.vector.tensor_mul(
                    o_final[h // 2][pb:pb + 64, 512 * J:512 * J + 512],
                    po[0:64, :], inv[:],
                )

            def attn_phase(J, fillers=()):
                fill = list(fillers)
                pending_norm = None
                for h in range(HPC):
                    qt = qk_sb[0 if h < 2 else 1]
                    kt = qk_sb[2 if h < 2 else 3]
                    pb = 64 * (h % 2)
                    tq0 = 512 * J
                    po = ps_acc.tile([65, 512], F32, tag="acc", name=f"po{h}_{J}")
                    n_i = 4 * J + 4

                    def emit_av(p_sb, i0):
                        for s in range(2):
                            i = i0 + s
                            d = max(0, 128 * (i - 4 * J))
                            nc.tensor.matmul(
                                po[:, d:512],
                                v_sb[i][:, VW * h:VW * h + VW],
                                p_sb[:, 512 * s + d:512 * s + 512],
                                start=(i == 0), stop=(i == n_i - 1),
                                skip_group_check=True,
                            )

                    pending_av = []  # AV lags AVLAG groups so PE never waits on exp
                    for i0 in range(0, n_i, 2):
                        ps = ps_big.tile([128, 1024], F32, tag="big", name=f"s{h}_{J}_{i0}")
                        for s in range(2):
                            i = i0 + s
                            nc.tensor.matmul(
                                ps[:, 512 * s:512 * s + 512],
                                kt[pb:pb + 64, 128 * i:128 * i + 128],
                                qt[pb:pb + 64, tq0:tq0 + 512],
                                start=True, stop=True,
                            )
                        p_sb = wp.tile([128, 1024], F32R, tag="p", bufs=2 + AVLAG,
                                       name=f"p{h}_{J}_{i0}")
                        nc.scalar.activation(
                            p_sb[:], ps[:], mybir.ActivationFunctionType.Exp,
                            scale=float(HD) ** -0.5,
                        )
                        for s in range(2):
                            i = i0 + s
                            if i >= 4 * J:  # diagonal tile: mask the triangle
                                d = 128 * (i - 4 * J)
                                nc.gpsimd.tensor_mul(
                                    p_sb[:, 512 * s + d:512 * s + d + 128],
                                    p_sb[:, 512 * s + d:512 * s + d + 128],
                                    tri_sb[:],
                                )
                        pending_av.append((p_sb, i0))
                        if len(pending_av) > AVLAG:
                            emit_av(*pending_av.pop(0))
                    for pa in pending_av:
                        emit_av(*pa)
                    if pending_norm is not None:
                        emit_normalize(*pending_norm)
                    pending_norm = (h, J, po)
                    if fill:
                        f = fill.pop(0)
                        if f is not None:
                            f()
                emit_normalize(*pending_norm)

            def out_proj(J, erange, alternate=False):
                for e in erange:
                    ps = ps_misc.tile([128, 512], F32, tag="misc", name=f"yp{J}_{e}")
                    for ct in range(2):
                        nc.tensor.matmul(
                            ps[:], ow_sb[ct][:, 128 * e:128 * e + 128],
                            o_final[ct][:, 512 * J:512 * J + 512],
                            start=(ct == 0), stop=(ct == 1),
                        )
                    y_sb = wp.tile([128, 512], F32, tag="ysb", bufs=2, name=f"y{J}_{e}")
                    if alternate and e % 2:
                        nc.scalar.copy(y_sb[:], ps[:])
                    else:
                        nc.vector.tensor_copy(y_sb[:], ps[:])
                    nc.sync.dma_start(
                        out=y_t[128 * e:128 * e + 128, 512 * J:512 * J + 512],
                        in_=y_sb[:],
                    )

            # ---------------- schedule ----------------
            with tc.tile_pool(name="xhi", bufs=1) as pxb:
                x_hi = [pxb.tile([128, 1024], F32R, name=f"xh{i}", tag=f"xh{i}")
                        for i in range(NKT)]
                with tc.tile_pool(name="xlo", bufs=1) as pxa:
                    x_lo = [pxa.tile([128, 1024], F32R, name=f"xl{i}", tag=f"xl{i}")
                            for i in range(NKT)]
                    # ---- DMA preamble, priority-ordered to the schedule's
                    # needs: w_qk + the first 512 token columns of x (s0) so
                    # attention J0 can start ~25us in, then w_v, then the
                    # second 512 columns (s1), then x_hi, then o_w.
                    for i in range(NKT):
                        nc.sync.dma_start(out=w_qk_sb[i][:], in_=w_qk[128 * i:128 * i + 128, :])
                        nc.sync.dma_start(out=x_lo[i][:, 0:512], in_=x_ls0_d[128 * i:128 * i + 128, :])
                        if i == 3:
                            # small tables, needed by the first RoPE
                            nc.sync.dma_start(out=bias_sb[:], in_=qk_bias[:, :])
                            nc.sync.dma_start(out=cs_sb[0:32, :], in_=cs_tab[:, :])
                            nc.sync.dma_start(out=sn_sb[0:64, :], in_=sn_tab[:, :])
                            for r in (32, 64, 96):
                                nc.sync.dma_start(out=cs_sb[r:r + 32, :], in_=cs_sb[0:32, :])
                            nc.sync.dma_start(out=sn_sb[64:128, :], in_=sn_sb[0:64, :])
                            nc.sync.dma_start(out=tri_sb[:], in_=tri[:, :])
                            nc.sync.dma_start(out=ones_sb[:1, :], in_=ones[:, :])
                            nc.sync.dma_start(out=wv_bias[:1, :], in_=w_v[C:C + 1, :])
                    for i in range(NKT):
                        nc.sync.dma_start(out=w_v_sb[i][:], in_=w_v[128 * i:128 * i + 128, :])
                    for i in range(NKT):
                        nc.sync.dma_start(out=x_lo[i][:, 512:1024], in_=x_ls1_d[128 * i:128 * i + 128, :])
                    for i in range(NKT):
                        nc.sync.dma_start(out=x_hi[i][:], in_=x_hi_d[128 * i:128 * i + 128, :])
                    for i in range(2):
                        nc.sync.dma_start(out=ow_sb[i][:], in_=o_wt[128 * i:128 * i + 128, :])

                    # pre-load the Exp activation table while Scalar is idle
                    warm = wp.tile([128, 2], F32, tag="inv", bufs=1, name="warm")
                    nc.scalar.activation(warm[:], tri_sb[:, 0:2],
                                         mybir.ActivationFunctionType.Exp)

                    # ---- P0a: QK projection for tokens 0..511 only (all of
                    # attention J0's needs), c-outer across four independent
                    # PSUM-bank chains in ps_misc/ps_acc; ps_big stays free
                    # for J0's score tiles.
                    pss = {
                        0: ps_misc.tile([128, 512], F32, tag="misc", name="qkp0_s0"),
                        2: ps_misc.tile([128, 512], F32, tag="misc", name="qkp2_s0"),
                        1: ps_acc.tile([128, 512], F32, tag="acc", name="qkp1_s0"),
                        3: ps_acc.tile([128, 512], F32, tag="acc", name="qkp3_s0"),
                    }
                    for c in range(NKT):
                        for m in (0, 2, 1, 3):
                            nc.tensor.matmul(
                                pss[m][:, :],
                                w_qk_sb[c][:, 128 * m:128 * m + 128],
                                x_lo[c][:, 0:512],
                                start=(c == 0), stop=(c == NKT - 1),
                                skip_group_check=True,
                            )
                    for m in (0, 2, 1, 3):
                        rope(m, 0, pss[m][:, :])
                    v_proj(x_lo, range(0, 4))

                    # fillers matched to the DMA arrival order: x_lo work in
                    # J0, x_hi work (qk1/vp) from J1 once x_hi has streamed in.
                    def qk1(m):
                        def f():
                            # keep the Tile scheduler from hoisting x_hi work
                            # ahead of attention J0 (x_hi streams in late)
                            with tc.tile_wait_until(0.034):
                                for s in range(2):
                                    qk_proj(m, x_hi, 1, s)
                        return f

                    def vpl(t0):
                        return lambda: v_proj(x_lo, range(t0, t0 + 2))

                    def vph(t0):
                        def f():
                            with tc.tile_wait_until(0.034):
                                v_proj(x_hi, range(t0, t0 + 2))
                        return f

                    def vph4():
                        def f():
                            with tc.tile_wait_until(0.034):
                                v_proj(x_hi, range(8, 12))
                        return f

                    def op(J, e0):
                        return lambda: out_proj(J, range(e0, e0 + 4))

                    def qk1v(m0, m1):
                        def f():
                            qk1(m0)()
                            qk1(m1)()
                        return f

                    attn_phase(0)
                    # ---- P0b: tokens 512..1023 (needed from J1 on), timed to
                    # overlap attention J0 while x_ls1 streams in.
                    for m in (0, 2, 1, 3):
                        qk_proj(m, x_lo, 0, 1)
                    v_proj(x_lo, range(4, 8))
                # x_lo freed
                attn_phase(1, [qk1(0), qk1(2), qk1v(1, 3), vph4()])
                attn_phase(2, [vph(12), vph(14), op(0, 0), op(0, 4)])
            attn_phase(3, [op(1, 0), op(1, 4), op(2, 0), op(2, 4)])
            out_proj(3, range(NKT), alternate=True)

    nc.finalize()
    return nc


def _round_tf32(x):
    b = np.ascontiguousarray(x, dtype=np.float32).view(np.uint32)
    lsb = (b >> np.uint32(13)) & np.uint32(1)
    r = (b + np.uint32(0x0FFF) + lsb) & np.uint32(0xFFFFE000)
    return r.view(np.float32)


_NC = None


def _perm128():
    """d order within a 2-head qk tile: per head [evens(32), odds(32)]."""
    ev = np.arange(0, 64, 2)
    od = np.arange(1, 64, 2)
    return np.concatenate([ev, od, 64 + ev, 64 + od])


def _host_inputs(x, freqs_cos, freqs_sin, qkv_w, qkv_b, o_w):
    """Build the 8 per-core input maps."""
    x = np.asarray(x, np.float32)
    qkv_w = np.asarray(qkv_w, np.float32)
    qkv_b = np.asarray(qkv_b, np.float32)
    o_w = np.asarray(o_w, np.float32)
    cosT = np.asarray(freqs_cos, np.float32).T  # [32, T]
    sinT = np.asarray(freqs_sin, np.float32).T
    cs = _round_tf32(cosT)  # [32, T]; replicated to 128 partitions on-device
    # per-head layout [evens(32), odds(32)]: odd quarters get -sin so rope
    # reduces to g*cs + quarterswap(g*sn)
    sn = _round_tf32(np.concatenate([sinT, -sinT]))  # [64, T]; replicated 2x
    p, g_idx = np.mgrid[0:128, 0:128]
    tri = (g_idx >= p).astype(np.float32)
    ones = np.ones((1, 128), np.float32)
    perm = _perm128()

    maps = []
    for core in range(NCORES):
        b, grp = divmod(core, NCORES // B)
        H0 = HPC * grp
        # permuted global q/k row indices for this core's heads (per 2-head tile)
        q_rows = np.concatenate([64 * (H0 + 2 * t) + perm for t in range(HPC // 2)])
        k_rows = C + q_rows
        v_rows = 2 * C + np.arange(64 * H0, 64 * H0 + DH)
        w_qk = _round_tf32(qkv_w[np.concatenate([q_rows, k_rows])].T)  # [C, 512]
        qk_bias = np.stack(
            [qkv_b[q_rows[:128]], qkv_b[q_rows[128:]],
             qkv_b[k_rows[:128]], qkv_b[k_rows[128:]]], axis=1)  # [128, 4]
        w_v = np.zeros((C + 1, HPC * VW), np.float32)
        for h in range(HPC):
            rows = v_rows[64 * h:64 * h + 64]
            w_v[:C, VW * h:VW * h + HD] = qkv_w[rows].T
            w_v[C, VW * h:VW * h + HD] = qkv_b[rows]
            w_v[C, VW * h + HD] = 1.0
        o_wt = _round_tf32(o_w[:, 64 * H0:64 * H0 + DH].T)  # [DH, C]
        xt = _round_tf32(x[b].T)
        maps.append({
            "x_ls0": np.ascontiguousarray(xt[:, 0:512]),
            "x_ls1": np.ascontiguousarray(xt[:, 512:1024]),
            "x_hi": np.ascontiguousarray(xt[:, 1024:2048]),
            "w_qk": w_qk,
            "qk_bias": qk_bias.astype(np.float32),
            "w_v": _round_tf32(w_v),
            "o_wt": o_wt,
            "cs_tab": cs,
            "sn_tab": sn,
            "tri": tri,
            "ones": ones,
        })
    return maps


def kernel(x, freqs_cos, freqs_sin, qkv_w, qkv_b, o_w, o_b, _trace=False, _tmpdir=None):
    global _NC
    if _NC is None:
        _NC = build_nc()
    maps = _host_inputs(x, freqs_cos, freqs_sin, qkv_w, qkv_b, o_w)
    kw = {}
    if _trace:
        kw = dict(trace=True, tmpdir=_tmpdir)
    res = run_bass_kernel_spmd(_NC, maps, list(range(NCORES)), **kw)
    o_b = np.asarray(o_b, np.float32)
    out = np.empty((B, T, C), np.float32)
    g = NCORES // B
    for b in range(B):
        acc = res.results[g * b]["y_t"].astype(np.float32)
        for j in range(1, g):
            acc = acc + res.results[g * b + j]["y_t"]
        out[b] = acc.T + o_b
    kernel._last = res
    return out
